# revision 1
# baseline (speedup 1.0000x reference)
"""Trainium2 Bass kernel for AxialMHA (B=2, N=2048, D=1024, H=16, dh=64).

Sharding: tensor-parallel over heads — 16 heads / 8 cores = 2 heads per core.
Each core computes q/k/v projections for its 2 heads (full batch), runs
attention, and produces a partial output projection (contraction over its
128 feature dims). Host sums the 8 partials and adds the effective bias
(bv @ Wproj + bproj — the v-bias commutes through softmax-weighted sums).

Device layout (per core):
  xT  [1024, 4096]  bf16  x transposed, d-major (shared by all cores)
  wq/wk/wv [1024, 128] bf16, wo [128, 1024] bf16, bq/bk [128, 1] f32
  out_p [1024, 4096] f32 partial projection (out-dim major)

Pipeline: QT/KT d-major via PE (moving = xT chunks, N=512); V token-major;
scores computed transposed (ST = K^T-tile vs Q, K=64 contraction, the two
heads row-packed on partition halves via tile_position); exp on ACT
(scale=1/8 folded in; logits are within +-2 so no max-subtraction is
needed); AV uses lhsT = [ones(64) | V_h] (M=128, same N-stream cost) so the
softmax denominators ride the AV matmul replicated across psum rows 0:64;
normalize = full-width DVE reciprocal + one DVE multiply; projection
partial at the end. Per-chunk Q/K/V tiles give the Tile scheduler fine
dependency granularity so the phases overlap.
"""

import os
import sys

import numpy as np
import ml_dtypes

for _p in ("/opt/trn_rl_repo",):
    if _p not in sys.path and os.path.isdir(_p):
        sys.path.insert(0, _p)

import concourse.bass as bass
import concourse.tile as tile
from concourse import bacc, mybir
from concourse.bass_utils import run_bass_kernel_spmd

BF16 = mybir.dt.bfloat16
F32 = mybir.dt.float32
AF = mybir.ActivationFunctionType

B, N, D, H, DH = 2, 2048, 1024, 16, 64
NC = 8            # cores
HC = H // NC      # heads per core = 2
TOK = B * N       # 4096
CH = 8            # token chunks of 512 for projections
CW = TOK // CH    # 512
KTD = D // 128    # 8 contraction tiles for projections
NKT = N // 128    # 16 ktok tiles per batch
QC = N // 512     # 4 qchunks per batch


def build_nc():
    nc = bacc.Bacc(
        "TRN2",
        target_bir_lowering=False,
        debug=False,
        enable_asserts=False,
        num_devices=NC,
    )
    xT = nc.dram_tensor("xT", [D, TOK], BF16, kind="ExternalInput").ap()
    wq = nc.dram_tensor("wq", [D, 128], BF16, kind="ExternalInput").ap()
    wk = nc.dram_tensor("wk", [D, 128], BF16, kind="ExternalInput").ap()
    wv = nc.dram_tensor("wv", [D, 128], BF16, kind="ExternalInput").ap()
    wo = nc.dram_tensor("wo", [128, D], BF16, kind="ExternalInput").ap()
    bq = nc.dram_tensor("bq", [128, 1], F32, kind="ExternalInput").ap()
    bk = nc.dram_tensor("bk", [128, 1], F32, kind="ExternalInput").ap()
    out_p = nc.dram_tensor("out_p", [D, TOK], F32, kind="ExternalOutput").ap()

    from contextlib import ExitStack

    with tile.TileContext(nc) as tc, ExitStack() as ctx:
        singles = ctx.enter_context(tc.tile_pool(name="singles", bufs=1))

        wq_sb = singles.tile([128, KTD, 128], BF16)
        nc.sync.dma_start(wq_sb, wq.rearrange("(ko p) m -> p ko m", p=128))
        wk_sb = singles.tile([128, KTD, 128], BF16)
        wv_sb = singles.tile([128, KTD, 128], BF16)
        bq_sb = singles.tile([128, 1], F32)
        bk_sb = singles.tile([128, 1], F32)
        wo_sb = singles.tile([128, D], BF16)

        # per-chunk tiles: fine dependency granularity lets attention start
        # as soon as the first projection chunk of a batch is done
        QT = [[singles.tile([128, CW], BF16, name=f"QT{b}_{q}") for q in range(QC)]
              for b in range(B)]
        KT = [[singles.tile([128, CW], BF16, name=f"KT{b}_{q}") for q in range(QC)]
              for b in range(B)]
        # token-major V, per head: cols 0:64 = ones (softmax denominators ride
        # the AV matmul as psum rows 0:64, replicated), cols 64:128 = V_h.
        V1 = [[singles.tile([128, 4, HC, 2 * DH], BF16, name=f"V1{b}_{q}")
               for q in range(QC)] for b in range(B)]
        for b in range(B):
            for q in range(QC):
                nc.vector.memset(V1[b][q], 1.0)
        yT = [singles.tile([128, N], BF16, name=f"yT{b}") for b in range(B)]

        # ---- Stage A: projections (emitted per batch) ----
        def stage_a(b, xpool, psA, psV):
            for cc in range(CH // B):
                c = b * (CH // B) + cc
                xt = xpool.tile([128, KTD, CW], BF16, tag="xt", name="xt")
                xs = xT[:, c * CW:(c + 1) * CW].rearrange("(ko p) n -> p ko n", p=128)
                nc.sync.dma_start(xt[:, 0:KTD // 2, :], xs[:, 0:KTD // 2, :])
                nc.sync.dma_start(xt[:, KTD // 2:, :], xs[:, KTD // 2:, :])
                if b == 0 and cc == 0:
                    # remaining weight/bias loads queue behind the first
                    # x-chunk so the first Q matmuls start sooner
                    nc.sync.dma_start(wk_sb, wk.rearrange("(ko p) m -> p ko m", p=128))
                    nc.sync.dma_start(wv_sb, wv.rearrange("(ko p) m -> p ko m", p=128))
                    nc.sync.dma_start(bq_sb, bq)
                    nc.sync.dma_start(bk_sb, bk)
                pq = psA.tile([128, CW], F32, tag="pqk", name="pq")
                for k in range(KTD):
                    nc.tensor.matmul(pq, lhsT=wq_sb[:, k, :], rhs=xt[:, k, :],
                                     start=(k == 0), stop=(k == KTD - 1))
                nc.vector.tensor_tensor(QT[b][cc], pq,
                                        bq_sb.to_broadcast((128, CW)),
                                        mybir.AluOpType.add)
                pk = psA.tile([128, CW], F32, tag="pqk", name="pk")
                for k in range(KTD):
                    nc.tensor.matmul(pk, lhsT=wk_sb[:, k, :], rhs=xt[:, k, :],
                                     start=(k == 0), stop=(k == KTD - 1))
                nc.vector.tensor_tensor(KT[b][cc], pk,
                                        bk_sb.to_broadcast((128, CW)),
                                        mybir.AluOpType.add)
                pv = psV.tile([128, 4, 128], F32, tag="pp", name="pv")
                for s in range(CW // 128):
                    for k in range(KTD):
                        nc.tensor.matmul(pv[:, s, :],
                                         lhsT=xt[:, k, s * 128:(s + 1) * 128],
                                         rhs=wv_sb[:, k, :],
                                         start=(k == 0), stop=(k == KTD - 1))
                for h in range(HC):
                    nc.vector.tensor_copy(V1[b][cc][:, :, h, DH:2 * DH],
                                          pv[:, :, h * DH:(h + 1) * DH])

        # ---- Output projection partial for one 512-token chunk ----
        def proj_chunk(b, cc, ppool, psV, psA, stp=None):
            cs = slice(cc * CW, (cc + 1) * CW)
            for ot in range(D // 128):
                pool_, tag_ = (psV, "pp") if ot % 2 == 0 else (psA, "pqk")
                pp = pool_.tile([128, CW], F32, tag=tag_, name="pp")
                nc.tensor.matmul(pp, lhsT=wo_sb[:, ot * 128:(ot + 1) * 128],
                                 rhs=yT[b][:, cs],
                                 start=True, stop=True)
                ps = ppool.tile([128, CW], F32, tag="ps", name="ps")
                nc.vector.tensor_copy(ps, pp)
                nc.sync.dma_start(
                    out_p[ot * 128:(ot + 1) * 128,
                          b * N + cc * CW:b * N + (cc + 1) * CW], ps)

        # ---- Attention (emitted per batch, optional proj interleave) ----
        def attention(b, stp, yps, epool, rpool, ppool, psV, psA, inline_proj):
            for qc in range(QC):
                qo = qc * 512
                py = [yps.tile([128, 512], F32, tag=f"y{h}", name=f"py{h}")
                      for h in range(HC)]
                for ktg in range(NKT // 2):
                    for h in range(HC):
                        hs = slice(h * DH, (h + 1) * DH)
                        stt = stp.tile([128, 2, 512], F32, tag="st", name="stt")
                        for j in range(2):
                            kt = ktg * 2 + j
                            kc, ks = divmod(kt, 4)
                            nc.tensor.matmul(
                                stt[:, j, :],
                                lhsT=KT[b][kc][hs, ks * 128:(ks + 1) * 128],
                                rhs=QT[b][qc][hs, :],
                                start=True, stop=True,
                                tile_position=(h * DH, 0),
                            )
                        et = epool.tile([128, 2, 512], BF16, tag="et", name="et")
                        nc.scalar.activation(et, stt, AF.Exp, scale=0.125)
                        for j in range(2):
                            kt = ktg * 2 + j
                            kc, ks = divmod(kt, 4)
                            nc.tensor.matmul(
                                py[h],
                                lhsT=V1[b][kc][:, ks, h, :],
                                rhs=et[:, j, :],
                                start=(ktg == 0 and j == 0),
                                stop=(ktg == NKT // 2 - 1 and j == 1),
                            )
                for h in range(HC):
                    rsb = rpool.tile([64, 512], F32, tag="rsb", name="rsb")
                    nc.vector.reciprocal(rsb, py[h][0:DH, :])
                    nc.vector.tensor_mul(
                        yT[b][h * DH:(h + 1) * DH, qo:qo + 512],
                        py[h][DH:2 * DH, :], rsb)
                if inline_proj:
                    proj_chunk(b, qc, ppool, psV, psA)

        with tc.tile_pool(name="xp", bufs=3) as xpool, \
             tc.tile_pool(name="psA", bufs=1, space="PSUM") as psA, \
             tc.tile_pool(name="psV", bufs=1, space="PSUM") as psV, \
             tc.tile_pool(name="stp", bufs=2, space="PSUM") as stp, \
             tc.tile_pool(name="yps", bufs=1, space="PSUM") as yps, \
             tc.tile_pool(name="ep", bufs=4) as epool, \
             tc.tile_pool(name="rp", bufs=4) as rpool, \
             tc.tile_pool(name="pp", bufs=4) as ppool:
            stage_a(0, xpool, psA, psV)
            nc.sync.dma_start(wo_sb, wo)
            stage_a(1, xpool, psA, psV)
            attention(0, stp, yps, epool, rpool, ppool, psV, psA, False)
            attention(1, stp, yps, epool, rpool, ppool, psV, psA, False)
            for b in range(B):
                for cc in range(QC):
                    proj_chunk(b, cc, ppool, psV, psA,
                               stp if b == B - 1 else None)

    nc.compile()
    return nc


_CACHE = {}


def _get_nc():
    if "nc" not in _CACHE:
        _CACHE["nc"] = build_nc()
    return _CACHE["nc"]


def _prep_inputs(x, Wqkv, bqkv):
    bf = ml_dtypes.bfloat16
    x = np.asarray(x, np.float32)
    Wqkv = np.asarray(Wqkv, np.float32)
    bqkv = np.asarray(bqkv, np.float32)
    xT = np.ascontiguousarray(x.reshape(TOK, D).T).astype(bf)
    in_maps = []
    for c in range(NC):
        cs = slice(c * 128, (c + 1) * 128)
        in_maps.append({
            "xT": xT,
            "wq": np.ascontiguousarray(Wqkv[:, 0 * D + c * 128:0 * D + (c + 1) * 128]).astype(bf),
            "wk": np.ascontiguousarray(Wqkv[:, 1 * D + c * 128:1 * D + (c + 1) * 128]).astype(bf),
            "wv": np.ascontiguousarray(Wqkv[:, 2 * D + c * 128:2 * D + (c + 1) * 128]).astype(bf),
            "wo": None,  # filled by caller (needs Wproj)
            "bq": np.ascontiguousarray(bqkv[0 * D + c * 128:0 * D + (c + 1) * 128]).reshape(128, 1).astype(np.float32),
            "bk": np.ascontiguousarray(bqkv[1 * D + c * 128:1 * D + (c + 1) * 128]).reshape(128, 1).astype(np.float32),
        })
    return in_maps


def _run(x, Wqkv, bqkv, Wproj, bproj, trace=False):
    bf = ml_dtypes.bfloat16
    Wproj = np.asarray(Wproj, np.float32)
    bproj = np.asarray(bproj, np.float32)
    bqkv_np = np.asarray(bqkv, np.float32)
    in_maps = _prep_inputs(x, Wqkv, bqkv_np)
    for c in range(NC):
        in_maps[c]["wo"] = np.ascontiguousarray(
            Wproj[c * 128:(c + 1) * 128, :]).astype(bf)
    nc = _get_nc()
    res = run_bass_kernel_spmd(nc, in_maps, core_ids=list(range(NC)), trace=trace)
    acc = res.results[0]["out_p"].astype(np.float32).copy()
    for c in range(1, NC):
        acc += res.results[c]["out_p"]
    bv = bqkv_np[2 * D:]
    bias_eff = (bv @ Wproj + bproj).astype(np.float32)
    out = np.ascontiguousarray(acc.T).reshape(B, N, D) + bias_eff
    return out.astype(np.float32), res


def kernel(x, Wqkv, bqkv, Wproj, bproj):
    out, _ = _run(x, Wqkv, bqkv, Wproj, bproj, trace=False)
    return out

